# revision 3
# baseline (speedup 1.0000x reference)
"""Trainium2 Bass kernel v2 for the 2-layer LSTM decoder + vocab projection.

Differences vs v1 (baseline):
  - LSTM1's x-part (emb@W, v@W, bias) is hoisted out of the recurrence into
    a full-width GEMM over positions, computed with fp8e4m3 DoubleRow
    matmuls (2x contraction throughput), pipelined 2 pos-tiles ahead of the
    recurrence, stored bf16 in an SBUF ring (gx).
  - Per step, gx is injected into the gates PSUM with an identity-column
    selection matmul (K=128, lhsT = ident[:, 32*tau:32*tau+32]) which also
    initializes the accumulation; the recurrent h-part matmuls (bf16,
    4-way column-tiled) accumulate on top.
  - The h2 transpose of step t-1 is deferred into step t, and the engine
    queues are ordered so eltwise waits are covered by gx/proj work (keeps
    the tensor engine continuously busy -> full clock).

Layouts (as v1): pos = t*32 + b; state partition = 32*c + b; gate free
layout 128*qslot + u with quarters ordered (i, f, o, g); "T" buffers
buf[u, c, pos] = x[pos, 128*c + u]. Vocab sharded 8 x 2048 rows.
"""

import numpy as np
import ml_dtypes

V, H, VS, KS = 16000, 512, 512, 512
N, T = 32, 128
NC = 8
VPAD = 16384
VSH = VPAD // NC
NPOS = N * T
BF16 = ml_dtypes.bfloat16
FP8 = ml_dtypes.float8_e4m3fn

S_X = 4.0     # fp8 scale on x (emb, values)
S_W = 64.0    # fp8 scale on W_ih1
GX_DESCALE = 1.0 / (S_X * S_W)

_QMAP = (0, 1, 3, 2)  # free-slot -> original quarter (i, f, o, g)


def _gate_cols(nH):
    j = np.arange(4)[:, None, None]
    qs = np.arange(4)[None, :, None]
    u = np.arange(128)[None, None, :]
    q = np.array(_QMAP)[qs]
    cols = nH * q + 128 * j + u
    return cols.reshape(4, 512)


def _gate_perm():
    """Flat permutation [2048]: gx free col f = 512*j + 128*qs + u ->
    original gate row 512*QMAP[qs] + 128*j + u (j-major blocks so the
    per-step selection matmul streams a contiguous 512 run)."""
    f = np.arange(2048)
    j, r = np.divmod(f, 512)
    qs, u = np.divmod(r, 128)
    return 512 * np.array(_QMAP)[qs] + 128 * j + u


def _prep_host(inputs):
    text = np.asarray(inputs["text"])
    values = np.asarray(inputs["values"], dtype=np.float32)
    emb = np.asarray(inputs["emb"], dtype=np.float32)

    text_tm = np.ascontiguousarray(text.T).reshape(-1)
    text_dev = np.ascontiguousarray(
        text_tm.reshape(NPOS // 128, 128).T).astype(np.int32)

    v = values.reshape(NPOS, VS)
    vTt = v.T.reshape(4, 128, NPOS).transpose(1, 0, 2)
    vT = np.ascontiguousarray(vTt).astype(BF16)
    vT8 = np.ascontiguousarray(vTt * S_X).astype(FP8)

    emb_bf = emb.astype(BF16)

    cols = _gate_cols(H)
    perm = _gate_perm()

    # W1x8: fp8 DR layout [128, 4(k2), 2(i), 2048]:
    #   W1x8[p, k2, i, f] = W_ih1[perm[f], 256*k2 + 128*i + p] * S_W
    W_ih1 = np.asarray(inputs["W_ih1"], dtype=np.float32)
    wsel = W_ih1[perm]                                 # [2048, 1024]
    w = wsel.T.reshape(4, 2, 128, 2048)                # [k2, i, p, f]
    W1x8 = np.ascontiguousarray(w.transpose(2, 0, 1, 3) * S_W).astype(FP8)

    b1 = (np.asarray(inputs["b_ih1"], np.float32)
          + np.asarray(inputs["b_hh1"], np.float32))
    b1rep = np.ascontiguousarray(
        np.broadcast_to(b1[perm][None, :], (128, 2048))).astype(BF16)

    def wh_layout(W):
        """W [2048, K] -> [128, K/128, 4, 512] bf16 (h-part rhs layout)."""
        sel = W[cols]                                  # [4, 512, K]
        K = W.shape[1]
        nk = K // 128
        ws = sel.transpose(2, 0, 1).reshape(nk, 128, 4, 512)
        return np.ascontiguousarray(ws.transpose(1, 0, 2, 3)).astype(BF16)

    W1h = wh_layout(np.asarray(inputs["W_hh1"], np.float32))   # [128,4,4,512]
    W2i = wh_layout(np.asarray(inputs["W_ih2"], np.float32))   # [128,4,4,512]
    W2h = wh_layout(np.asarray(inputs["W_hh2"], np.float32))   # [128,4,4,512]
    b2 = (np.asarray(inputs["b_ih2"], np.float32)
          + np.asarray(inputs["b_hh2"], np.float32))
    b2rep = np.ascontiguousarray(np.broadcast_to(
        b2[cols][None] / 128.0, (128, 4, 512))).astype(BF16)

    W_out = np.asarray(inputs["W_out"], dtype=np.float32)
    b_out = np.asarray(inputs["b_out"], dtype=np.float32)
    Wp = np.zeros((VPAD, KS + VS), dtype=np.float32)
    Wp[:V] = W_out
    bp = np.zeros(VPAD, dtype=np.float32)
    bp[:V] = b_out

    shared = {"text_dev": text_dev, "vT": vT, "vT8": vT8, "emb": emb_bf,
              "W1x8": W1x8, "b1rep": b1rep, "W1h": W1h, "W2i": W2i,
              "W2h": W2h, "b2rep": b2rep}
    per_core = []
    for c in range(NC):
        wsh = Wp[c * VSH:(c + 1) * VSH]
        WoT = np.ascontiguousarray(
            wsh.T.reshape(8, 128, VSH).transpose(1, 0, 2)).astype(BF16)
        bo = np.ascontiguousarray(
            bp[c * VSH:(c + 1) * VSH].reshape(16, 128).T)
        per_core.append({"WoT": WoT, "bo": bo})
    return shared, per_core


def _build(t_steps=T, debug=False):
    import concourse.bacc as bacc
    import concourse.bass as bass
    import concourse.mybir as mybir
    import concourse.tile as tile
    from concourse.masks import make_identity

    fp32 = mybir.dt.float32
    bf16 = mybir.dt.bfloat16
    fp8 = mybir.dt.float8e4
    AF = mybir.ActivationFunctionType
    DR = mybir.MatmulPerfMode.DoubleRow

    nc = bacc.Bacc("TRN2", target_bir_lowering=False, debug=False,
                   num_devices=NC)

    d_text = nc.declare_dram_parameter("text_dev", [128, 32], mybir.dt.int32,
                                       isOutput=False)
    d_emb = nc.declare_dram_parameter("emb", [V, H], bf16, isOutput=False)
    d_vT = nc.declare_dram_parameter("vT", [128, 4, NPOS], bf16,
                                     isOutput=False)
    d_vT8 = nc.declare_dram_parameter("vT8", [128, 4, NPOS], fp8,
                                      isOutput=False)
    d_W1x8 = nc.declare_dram_parameter("W1x8", [128, 4, 2, 2048], fp8,
                                       isOutput=False)
    d_b1rep = nc.declare_dram_parameter("b1rep", [128, 2048], bf16,
                                        isOutput=False)
    d_W1h = nc.declare_dram_parameter("W1h", [128, 4, 4, 512], bf16,
                                      isOutput=False)
    d_W2i = nc.declare_dram_parameter("W2i", [128, 4, 4, 512], bf16,
                                      isOutput=False)
    d_W2h = nc.declare_dram_parameter("W2h", [128, 4, 4, 512], bf16,
                                      isOutput=False)
    d_b2rep = nc.declare_dram_parameter("b2rep", [128, 4, 512], bf16,
                                        isOutput=False)
    d_WoT = nc.declare_dram_parameter("WoT", [128, 8, VSH], bf16,
                                      isOutput=False)
    d_bo = nc.declare_dram_parameter("bo", [128, 16], fp32, isOutput=False)
    d_out = nc.declare_dram_parameter("out", [VSH, NPOS], fp32, isOutput=True)
    d_h1dbg = d_h2dbg = d_gxdbg = None
    if debug:
        d_h1dbg = nc.declare_dram_parameter(
            "h1dbg", [128, t_steps * 128], bf16, isOutput=True)
        d_h2dbg = nc.declare_dram_parameter(
            "h2dbg", [128, t_steps * 128], bf16, isOutput=True)
        d_gxdbg = nc.declare_dram_parameter(
            "gxdbg", [128, ((t_steps + 3) // 4) * 2048], bf16, isOutput=True)

    n_tiles = (t_steps * 32 + 127) // 128  # pos-tiles of 128

    with tile.TileContext(nc) as tc:
        with (
            tc.tile_pool(name="persist", bufs=1) as persist,
            tc.tile_pool(name="gather", bufs=3) as gpool,
            tc.tile_pool(name="embT", bufs=4) as epool,
            tc.tile_pool(name="gx", bufs=3) as gxpool,
            tc.tile_pool(name="state", bufs=2) as spool,
            tc.tile_pool(name="work", bufs=3) as wpool,
            tc.tile_pool(name="psg", bufs=1, space="PSUM") as psg,
            tc.tile_pool(name="pst", bufs=3, space="PSUM") as pst,
            tc.tile_pool(name="pgx", bufs=1, space="PSUM") as pgxp,
            tc.tile_pool(name="proj_w", bufs=2) as projw,
            tc.tile_pool(name="proj_o", bufs=2) as projo,
            tc.tile_pool(name="psp", bufs=2, space="PSUM") as psp,
        ):
            # ---- static tiles ----
            # DMA order matters: boot-critical first (txt -> gathers,
            # W1x8/vT8 -> gx GEMM, W1h -> first l1h), big late-use last.
            txt = persist.tile([128, 32], mybir.dt.int32)
            nc.sync.dma_start(txt[:], d_text[:])
            W1x8 = persist.tile([128, 4, 2, 2048], fp8)
            nc.sync.dma_start(W1x8[:], d_W1x8[:])
            vT8 = persist.tile([128, 4, NPOS], fp8)
            nc.sync.dma_start(vT8[:], d_vT8[:])
            W1h = persist.tile([128, 4, 4, 512], bf16)
            nc.sync.dma_start(W1h[:], d_W1h[:])
            b1rep = persist.tile([128, 2048], bf16)
            nc.sync.dma_start(b1rep[:], d_b1rep[:])
            b2rep = persist.tile([128, 4, 512], bf16)
            nc.sync.dma_start(b2rep[:], d_b2rep[:])
            W2h = persist.tile([128, 4, 4, 512], bf16)
            nc.sync.dma_start(W2h[:], d_W2h[:])
            W2i = persist.tile([128, 4, 4, 512], bf16)
            nc.sync.dma_start(W2i[:], d_W2i[:])
            vT = persist.tile([128, 4, NPOS], bf16)
            nc.sync.dma_start(vT[:], d_vT[:])
            bo = persist.tile([128, 16], fp32)
            nc.sync.dma_start(bo[:], d_bo[:])

            ident = persist.tile([128, 128], bf16)
            make_identity(nc, ident[:])
            ones32 = persist.tile([128, 32], bf16)
            nc.gpsimd.memset(ones32[:], 1.0)

            h2T_buf = persist.tile([128, 4, NPOS], bf16)

            h1T_prev = spool.tile([128, 128], bf16, tag="h1T")
            nc.gpsimd.memset(h1T_prev[:], 0.0)
            h2T_init = persist.tile([128, 128], bf16)
            nc.gpsimd.memset(h2T_init[:], 0.0)
            c1_prev = spool.tile([128, 128], fp32, tag="c1")
            nc.gpsimd.memset(c1_prev[:], 0.0)
            c2_prev = spool.tile([128, 128], fp32, tag="c2")
            nc.gpsimd.memset(c2_prev[:], 0.0)

            # ---- gx pipeline pieces ----
            embT_tiles = {}
            gx_tiles = {}
            gather_tiles = {}
            pt_chunks = {}

            def issue_gather(tt):
                if tt >= n_tiles or tt in gather_tiles:
                    return
                g = gpool.tile([128, H], bf16, tag="embg")
                nc.gpsimd.indirect_dma_start(
                    out=g[:], out_offset=None, in_=d_emb[:],
                    in_offset=bass.IndirectOffsetOnAxis(
                        ap=txt[:, tt:tt + 1], axis=0))
                gather_tiles[tt] = g

            def transpose_chunk_pe(tt, c):
                """PE phase: transpose chunk c of gathered tile tt."""
                if tt >= n_tiles:
                    return
                g = gather_tiles[tt]
                pt = pst.tile([128, 128], bf16, tag="tp")
                nc.tensor.transpose(pt[:], g[:, 128 * c:128 * (c + 1)],
                                    ident[:])
                pt_chunks[(tt, c)] = pt

            def transpose_chunk_act(tt, c):
                """Act phase: copy transposed chunk into embT8 (fp8, *S_X)."""
                if tt >= n_tiles:
                    return
                if tt not in embT_tiles:
                    embT_tiles[tt] = epool.tile([128, 4, 128], fp8,
                                                tag="embT8", name="embT8")
                pt = pt_chunks.pop((tt, c))
                nc.scalar.activation(embT_tiles[tt][:, c, :], pt[:],
                                     AF.Identity, scale=S_X)

            def gx_block_pe(tt, nb):
                """PE phase of gx GEMM block nb for pos-tile tt."""
                if tt >= n_tiles:
                    return None
                et = embT_tiles[tt]
                pgx = pgxp.tile([128, 512], fp32, tag="pgx")
                for k2 in range(4):
                    lhsT = (et[:, 2 * k2:2 * k2 + 2, :] if k2 < 2 else
                            vT8[:, 2 * (k2 - 2):2 * (k2 - 2) + 2,
                                128 * tt:128 * (tt + 1)])
                    nc.tensor.matmul(
                        pgx[:], lhsT,
                        W1x8[:, k2, :, 512 * nb:512 * (nb + 1)],
                        start=(k2 == 0), stop=(k2 == 3),
                        skip_group_check=True, perf_mode=DR)
                return pgx

            def gx_block_copy(tt, nb, pgx):
                """DVE phase: descale-copy gx psum block into the bf16 ring."""
                if pgx is None:
                    return
                if tt not in gx_tiles:
                    gx_tiles[tt] = gxpool.tile([128, 2048], bf16, tag="gxt",
                                               name="gxt")
                nc.vector.scalar_tensor_tensor(
                    gx_tiles[tt][:, 512 * nb:512 * (nb + 1)], pgx[:],
                    GX_DESCALE, b1rep[:, 512 * nb:512 * (nb + 1)],
                    op0=mybir.AluOpType.mult, op1=mybir.AluOpType.add)

            def hpart(g, lhs_fn, W, stop=False):
                for k in range(4):
                    lhs = lhs_fn(k)
                    for j in range(4):
                        nc.tensor.matmul(
                            g[32 * j:32 * (j + 1), :], lhs, W[:, k, j, :],
                            start=False,
                            stop=(stop and k == 3 and j == 3),
                            skip_group_check=True, tile_position=(0, 32 * j))

            def emit_proj(pt_, vt, idx, nk=8, k0=0, pw=512):
                if k0 == 0:
                    wo = projw.tile([128, 8, 128], bf16, tag="wo")
                    nc.sync.dma_start(wo[:],
                                      d_WoT[:, :, 128 * vt:128 * (vt + 1)])
                    emit_proj.wo = wo
                    emit_proj.ps = psp.tile([128, 512], fp32, tag="pp")
                wo, ps = emit_proj.wo, emit_proj.ps
                for k in range(k0, k0 + nk):
                    rhs = (h2T_buf[:, k, 512 * pt_:512 * pt_ + pw] if k < 4
                           else vT[:, k - 4, 512 * pt_:512 * pt_ + pw])
                    nc.tensor.matmul(ps[:, :pw], wo[:, k, :], rhs,
                                     start=(k == 0), stop=(k == 7),
                                     skip_group_check=True)
                if k0 + nk < 8:
                    return
                ot = projo.tile([128, 512], fp32, tag="ot")
                if idx % 2 == 0:
                    nc.scalar.activation(ot[:, :pw], ps[:, :pw], AF.Identity,
                                         bias=bo[:, vt:vt + 1])
                else:
                    nc.vector.scalar_tensor_tensor(
                        ot[:, :pw], ps[:, :pw], 1.0,
                        bo[:, vt:vt + 1].to_broadcast([128, pw]),
                        op0=mybir.AluOpType.mult,
                        op1=mybir.AluOpType.add)
                nc.sync.dma_start(
                    d_out[128 * vt:128 * (vt + 1), 512 * pt_:512 * pt_ + pw],
                    ot[:, :pw])

            # ---- boot: gathers 0-3, transposes 0-2, gx tiles 0-1 ----
            for tt in range(min(4, n_tiles)):
                issue_gather(tt)
            for tt in range(min(3, n_tiles)):
                for c in range(4):
                    transpose_chunk_pe(tt, c)
                    transpose_chunk_act(tt, c)
            for tt in range(min(2, n_tiles)):
                for nb in range(4):
                    gx_block_copy(tt, nb, gx_block_pe(tt, nb))
            if debug:
                for tt in range(min(2, n_tiles)):
                    nc.sync.dma_start(
                        d_gxdbg[:, 2048 * tt:2048 * (tt + 1)],
                        gx_tiles[tt][:])

            h2_prev = None  # (h2 tile, step) awaiting transpose

            def defer_h2_transpose():
                """Transpose h2 of the previous step (PE) — Act copy follows
                later via h2_flush_act."""
                if h2_prev is None:
                    return None
                h2, tprev = h2_prev
                pt2 = pst.tile([128, 128], bf16, tag="tp")
                nc.tensor.transpose(pt2[:], h2[:], ident[:])
                return (pt2, tprev)

            def h2_flush_act(pending):
                if pending is None:
                    return
                pt2, tprev = pending
                nc.scalar.copy(h2T_buf[:, :, 32 * tprev:32 * (tprev + 1)],
                               pt2[:].rearrange("p (c b) -> p c b", c=4))

            # ---- recurrence ----
            for t in range(t_steps):
                tau = t % 4
                tt = t // 4
                u = t - 16

                # [PE] sel-inject gx (starts g1 accumulation; j-major gx)
                g1 = psg.tile([128, 512], fp32, tag="g1")
                for j in range(4):
                    nc.tensor.matmul(
                        g1[32 * j:32 * (j + 1), :],
                        ident[:, 32 * tau:32 * tau + 32],
                        gx_tiles[tt][:, 512 * j:512 * (j + 1)],
                        start=True, stop=False, skip_group_check=True,
                        tile_position=(0, 32 * j))
                # [PE] l1h
                hpart(g1, lambda k: h1T_prev[:, 32 * k:32 * (k + 1)],
                      W1h[:], stop=True)

                # [Act] eltwise1 part 1; [DVE] part 2
                sig1 = wpool.tile([128, 384], fp32, tag="sig1")
                nc.scalar.activation(sig1[:], g1[:, 0:384], AF.Sigmoid)
                tg1 = wpool.tile([128, 128], fp32, tag="tg1")
                nc.scalar.activation(tg1[:], g1[:, 384:512], AF.Tanh)
                t2a = wpool.tile([128, 128], fp32, tag="t2a")
                nc.vector.tensor_mul(t2a[:], sig1[:, 128:256], c1_prev[:])
                t1a = wpool.tile([128, 128], fp32, tag="t1a")
                nc.vector.tensor_mul(t1a[:], sig1[:, 0:128], tg1[:])
                c1_new = spool.tile([128, 128], fp32, tag="c1")
                nc.vector.tensor_add(c1_new[:], t1a[:], t2a[:])

                # [PE] g2 bias (K=128 full-bandwidth: ones32 x b2rep/128)
                g2 = psg.tile([128, 512], fp32, tag="g2")
                for j in range(4):
                    nc.tensor.matmul(
                        g2[32 * j:32 * (j + 1), :], ones32[:, 0:32],
                        b2rep[:, j, :], start=True, stop=False,
                        skip_group_check=True, tile_position=(0, 32 * j))

                # [PE] gx pipeline fillers
                if tau == 0:
                    issue_gather(tt + 4)
                transpose_chunk_pe(tt + 3, tau)
                pgx_pend = gx_block_pe(tt + 2, tau)

                # [PE] deferred h2 transpose of t-1
                pending_h2 = defer_h2_transpose()

                # [Act] tanh(c1); [DVE] h1
                tc1 = wpool.tile([128, 128], fp32, tag="tc1")
                nc.scalar.activation(tc1[:], c1_new[:], AF.Tanh)
                h1 = wpool.tile([128, 128], bf16, tag="h1")
                nc.vector.tensor_mul(h1[:], sig1[:, 256:384], tc1[:])
                # [Act] h2T_buf copy of t-1, then embT8 copy
                h2_flush_act(pending_h2)
                transpose_chunk_act(tt + 3, tau)

                # [PE] proj half a
                if t_steps == T and u >= 0:
                    emit_proj(u // 16, u % 16, u, nk=3, k0=0)

                # [PE] l2h2 (uses h2T_buf slice of t-1)
                if t == 0:
                    hpart(g2, lambda k: h2T_init[:, 32 * k:32 * (k + 1)],
                          W2h[:])
                else:
                    hpart(g2,
                          lambda k: h2T_buf[:, k, 32 * (t - 1):32 * t],
                          W2h[:])

                # [PE] transpose h1 -> h1T; [DVE] copy to SBUF
                pt1 = pst.tile([128, 128], bf16, tag="tp")
                nc.tensor.transpose(pt1[:], h1[:], ident[:])
                h1T = spool.tile([128, 128], bf16, tag="h1T")
                nc.vector.tensor_copy(h1T[:], pt1[:])

                # [PE] l2h1 (needs h1T of this step)
                hpart(g2, lambda k: h1T[:, 32 * k:32 * (k + 1)],
                      W2i[:], stop=True)

                # [PE] proj half b
                if t_steps == T and u >= 0:
                    emit_proj(u // 16, u % 16, u, nk=5, k0=3)

                # eltwise lstm2
                sig2 = wpool.tile([128, 384], fp32, tag="sig2")
                nc.scalar.activation(sig2[:], g2[:, 0:384], AF.Sigmoid)
                tg2 = wpool.tile([128, 128], fp32, tag="tg2")
                nc.scalar.activation(tg2[:], g2[:, 384:512], AF.Tanh)
                t2b = wpool.tile([128, 128], fp32, tag="t2b")
                nc.vector.tensor_mul(t2b[:], sig2[:, 128:256], c2_prev[:])
                t1b = wpool.tile([128, 128], fp32, tag="t1b")
                nc.vector.tensor_mul(t1b[:], sig2[:, 0:128], tg2[:])
                c2_new = spool.tile([128, 128], fp32, tag="c2")
                nc.vector.tensor_add(c2_new[:], t1b[:], t2b[:])
                # [DVE] gx psum copy (late: keeps it off the eltwise path)
                gx_block_copy(tt + 2, tau, pgx_pend)
                if debug and tau == 3 and (tt + 2) < n_tiles:
                    nc.sync.dma_start(
                        d_gxdbg[:, 2048 * (tt + 2):2048 * (tt + 3)],
                        gx_tiles[tt + 2][:])
                tc2 = wpool.tile([128, 128], fp32, tag="tc2")
                nc.scalar.activation(tc2[:], c2_new[:], AF.Tanh)
                h2 = wpool.tile([128, 128], bf16, tag="h2")
                nc.vector.tensor_mul(h2[:], sig2[:, 256:384], tc2[:])

                if debug:
                    nc.sync.dma_start(d_h1dbg[:, 128 * t:128 * (t + 1)],
                                      h1[:])
                    nc.sync.dma_start(d_h2dbg[:, 128 * t:128 * (t + 1)],
                                      h2[:])

                h2_prev = (h2, t)
                h1T_prev, c1_prev, c2_prev = h1T, c1_new, c2_new

            # final h2 transpose + copy
            h2_flush_act(defer_h2_transpose())

            # ---- remaining projection ----
            n_pt = (t_steps * 32 + 511) // 512
            if t_steps == T:
                for vt in range(VSH // 128):
                    emit_proj(n_pt - 1, vt, vt)
            else:
                for vt in range(VSH // 128):
                    for pt_ in range(n_pt):
                        pw = min(512, t_steps * 32 - 512 * pt_)
                        emit_proj(pt_, vt, vt + pt_, pw=pw)

    nc.compile()
    return nc


_CACHE = {}


def _get_nc(t_steps=T, debug=False):
    key = (t_steps, debug)
    if key not in _CACHE:
        _CACHE[key] = _build(t_steps, debug)
    return _CACHE[key]


def kernel(**inputs):
    from concourse.bass_utils import run_bass_kernel_spmd

    shared, per_core = _prep_host(inputs)
    nc = _get_nc(T)
    in_maps = []
    for c in range(NC):
        m = dict(shared)
        m.update(per_core[c])
        in_maps.append(m)
    res = run_bass_kernel_spmd(nc, in_maps, list(range(NC)))
    cat = np.concatenate([res.results[c]["out"] for c in range(NC)], axis=0)
    cat = cat[:V]
    out = cat.reshape(V, T, N).transpose(2, 1, 0)
    return np.ascontiguousarray(out.astype(np.float32))
